# revision 32
# baseline (speedup 1.0000x reference)
"""Trainium2 Bass kernel for MinEuclideanDistBlock.

Math (per batch b):
  d2[c,w,k] = ||x[b,c,w:w+S]||^2 + ||sh[c,k]||^2 - 2 <x[b,c,w:w+S], sh[c,k]>
  out[b,k]  = min_w  sum_c sqrt(d2[c,w,k])

Kernel strategy (per core, data-parallel over batch: 16 of 128 batches):
  - One matmul group per (b,c) produces (sqw - 2*dot) in PSUM via a 51-row
    contraction: 50 im2col rows of x against lhsT rows -2*sh[c,:,s]^T, plus
    an sqw row paired with a ones lhsT row.  ||sh||^2 is folded into the
    sqrt activation as a per-partition bias, so dist = Sqrt(psum + sq_s).
  - x is cast-loaded to bf16 (SWDGE) and bounced to a DRAM scratch; im2col
    rhs tiles are then HWDGE DMAs straight from DRAM, one ~800 KB DMA per
    (channel, 4-batch group).  Reading from DRAM avoids the
    single-SBUF-partition source port bottleneck that a SBUF->SBUF im2col
    hits (~27 GB/s effective).
  - matmul operands are bf16 (the PE streams 512-col chunks at its
    1 column/cycle issue rate either way; bf16 keeps the moving-operand
    byte volume down), PSUM accumulates f32.
  - sq_w (sliding-window sum of squares) comes from a prefix-sum
    (tensor_tensor_scan) and a shifted subtract.
  - channel reduce: two f16 tensor_adds (DVE 2x mode; dist tiles padded to
    2000 cols for 4B alignment) and a min tensor_reduce per batch.
"""

import numpy as np
from contextlib import ExitStack

import concourse.bass as bass
import concourse.bacc as bacc
import concourse.mybir as mybir
import concourse.tile as tile
from concourse import masks
from concourse.bass_utils import run_bass_kernel_spmd

B, C, L = 128, 3, 2048
K, S = 128, 50
W = L - S + 1  # 1999
NCORES = 8
BPC = B // NCORES  # batches per core
BC = BPC * C  # x rows per core
CONTRACT = S + 1  # 50 shapelet rows + sq_w row
QB = 4  # batches per im2col DMA group
NQ = BPC // QB

F32 = mybir.dt.float32
F16 = mybir.dt.float16
BF16 = mybir.dt.bfloat16
ACT = mybir.ActivationFunctionType
ALU = mybir.AluOpType
AXIS = mybir.AxisListType
WP = 2000  # padded W so f16 tile rows stay 4B-aligned (DVE 2x mode)

LAST_RESULTS = None  # BassKernelResults of the last run (for test harness)


def _body(ctx, tc, out_ap, x_ap, sh_ap):
    nc = tc.nc

    const = ctx.enter_context(tc.tile_pool(name="const", bufs=1))
    ident = const.tile([128, 128], F32)
    masks.make_identity(nc, ident[:])
    sqs = const.tile([K, C], F32)
    ones_row = const.tile([1, K], BF16)
    nc.vector.memset(ones_row[:], 1.0)
    lhsT = []
    for c in range(C):
        lhsT.append(const.tile([CONTRACT, K], BF16, tag=f"lhsT{c}", name=f"lhsT{c}"))

    persist = ctx.enter_context(tc.tile_pool(name="persist", bufs=1))
    x_b = persist.tile([BC, L], BF16)
    sqw_b = persist.tile([BC, WP], BF16)
    res = persist.tile([K, BPC], F32)
    dram = ctx.enter_context(tc.tile_pool(name="dram", bufs=1, space="DRAM"))
    xh_dram = dram.tile([BC, L], BF16)

    # cast-load x straight to bf16 (SWDGE) and bounce it to DRAM for im2col
    nc.gpsimd.dma_start(x_b[:], x_ap.rearrange("b c l -> (b c) l"))
    nc.sync.dma_start(xh_dram[:], x_b[:])

    # ---- prep: sq_w tree (fp16) + shapelet transforms ----
    with (
        tc.tile_pool(name="prep", bufs=1) as prep,
        tc.tile_pool(name="prep_ps", bufs=1, space="PSUM") as prep_ps,
    ):
        # shapelet prep first: its small DVE ops must not queue behind the
        # long scan on the DVE queue, so lhsT is ready before q0's rhs lands
        for c in range(C):
            sh_raw = prep.tile([K, S], F32, tag="sh_raw")
            nc.scalar.dma_start(sh_raw[:], sh_ap[c])
            sh_m2 = prep.tile([K, S], F32, tag="sh_m2")
            nc.vector.tensor_scalar_mul(sh_m2[:], sh_raw[:], -2.0)
            sh_sq = prep.tile([K, S], F32, tag="sh_sq")
            nc.vector.tensor_mul(sh_sq[:], sh_raw[:], sh_raw[:])
            nc.vector.reduce_sum(sqs[:, c : c + 1], sh_sq[:], axis=AXIS.X)
            pt = prep_ps.tile([S, K], F32, tag="pt")
            nc.tensor.transpose(pt[:], sh_m2[:], ident[:])
            nc.scalar.activation(lhsT[c][:S, :], pt[:], ACT.Copy)
            nc.scalar.dma_start(lhsT[c][S : S + 1, :], ones_row[:])

        # sliding sum-of-squares via prefix scan: sqw[w] = csum[w+S-1]-csum[w-1]
        # (chunked so Square/scan pipeline: scan chunk 1 chains via `initial`)
        H = L // 2
        xsq = prep.tile([BC, L], F16)
        csum = prep.tile([BC, L], F32)
        nc.scalar.activation(xsq[:, :H], x_b[:, :H], ACT.Square)
        nc.vector.tensor_tensor_scan(
            csum[:, :H], xsq[:, :H], xsq[:, :H], 0.0, op0=ALU.add, op1=ALU.bypass
        )
        nc.scalar.activation(xsq[:, H:], x_b[:, H:], ACT.Square)
        nc.vector.tensor_tensor_scan(
            csum[:, H:],
            xsq[:, H:],
            xsq[:, H:],
            csum[:, H - 1 : H],
            op0=ALU.add,
            op1=ALU.bypass,
        )
        nc.vector.tensor_scalar_add(sqw_b[:, 0:1], csum[:, S - 1 : S], 0.0)
        nc.vector.tensor_sub(
            sqw_b[:, 1:W], csum[:, S : S + W - 1], csum[:, 0 : W - 1]
        )

    # ---- main loop ----
    rhsp = ctx.enter_context(tc.tile_pool(name="rhs", bufs=2))
    psum = ctx.enter_context(tc.tile_pool(name="mm", bufs=2, space="PSUM"))
    distp = ctx.enter_context(tc.tile_pool(name="dist", bufs=3))

    nchunk = (W + 511) // 512
    xh_base = xh_dram[:]
    for q in range(NQ):
        rhs_q = []
        for c in range(C):
            rhs = rhsp.tile(
                [CONTRACT, QB * W], BF16, tag=f"rhs{c}", name=f"rhs{c}_{q}"
            )
            # im2col from the bf16 DRAM bounce.  During the ramp (q=0) split
            # channels across both HWDGE rings and keep the tree-gated sqw
            # rows off the im2col ring so nothing serializes behind them.
            src = bass.AP(
                xh_base.tensor,
                xh_base.offset + (QB * q * C + c) * L,
                [[1, S], [C * L, QB], [1, W]],
            )
            im2col_eng = nc.scalar if (q == 0 and c == 1) else nc.sync
            im2col_eng.dma_start(rhs[:S, :], src)
            # sqw row: src[i, w] = sqw_b[(4q+i)*C + c, w]
            sqw_base = sqw_b[:]
            sqw_src = bass.AP(
                sqw_base.tensor,
                sqw_base.offset + (QB * q * C + c) * sqw_base.ap[0][0],
                [[C * sqw_base.ap[0][0], QB], [1, W]],
            )
            sqw_eng = nc.scalar if q == 0 else nc.sync
            sqw_eng.dma_start(rhs[S : S + 1, :], sqw_src)
            rhs_q.append(rhs)

        for i in range(QB):
            b = QB * q + i
            dist = []
            for c in range(C):
                d2 = psum.tile([K, 2048], F32, tag="d2")
                for ch in range(nchunk):
                    w0 = 512 * ch
                    cw = min(512, W - w0)
                    nc.tensor.matmul(
                        d2[:, w0 : w0 + cw],
                        lhsT[c][:],
                        rhs_q[c][:, i * W + w0 : i * W + w0 + cw],
                        start=True,
                        stop=True,
                    )
                dt_ = distp.tile([K, WP], F16, tag=f"dist{c}", name=f"dist{c}")
                nc.scalar.activation(
                    dt_[:, :W], d2[:, :W], ACT.Sqrt, bias=sqs[:, c : c + 1]
                )
                dist.append(dt_)
            t01 = distp.tile([K, WP], F16, tag="t01")
            nc.vector.tensor_add(t01[:, :W], dist[0][:, :W], dist[1][:, :W])
            scr = distp.tile([K, WP], F16, tag="scr")
            nc.vector.tensor_add(scr[:, :W], t01[:, :W], dist[2][:, :W])
            nc.vector.tensor_reduce(
                res[:, b : b + 1], scr[:, :W], axis=AXIS.X, op=ALU.min
            )

    # ---- transpose result (K, BPC) -> (BPC, K) and store ----
    rt = psum.tile([BPC, K], F32, tag="d2")
    nc.tensor.transpose(rt[:], res[:], ident[:])
    out_sb = distp.tile([BPC, K], F32, tag="out_sb")
    nc.scalar.activation(out_sb[:], rt[:], ACT.Copy)
    nc.sync.dma_start(out_ap, out_sb[:])


def _build():
    nc = bacc.Bacc(
        "TRN2", target_bir_lowering=False, debug=False, num_devices=NCORES
    )
    x = nc.dram_tensor("x", [BPC, C, L], F32, kind="ExternalInput").ap()
    sh = nc.dram_tensor("sh", [C, K, S], F32, kind="ExternalInput").ap()
    out = nc.dram_tensor("out", [BPC, K], F32, kind="ExternalOutput").ap()
    with tile.TileContext(nc) as tc, ExitStack() as ctx:
        _body(ctx, tc, out, x, sh)
    nc.compile()
    return nc


def kernel(x, shapelets, trace=False):
    global LAST_RESULTS
    x = np.ascontiguousarray(np.asarray(x, dtype=np.float32))
    shapelets = np.ascontiguousarray(np.asarray(shapelets, dtype=np.float32))
    nc = _build()
    in_maps = [
        {"x": x[i * BPC : (i + 1) * BPC], "sh": shapelets} for i in range(NCORES)
    ]
    results = run_bass_kernel_spmd(
        nc, in_maps, core_ids=list(range(NCORES)), trace=trace
    )
    LAST_RESULTS = results
    out = np.concatenate([results.results[i]["out"] for i in range(NCORES)], axis=0)
    return out.reshape(B, 1, K)
